# revision 39
# baseline (speedup 1.0000x reference)
"""3-layer GAT on Trainium2, 8 NeuronCores, full computation on device.

Sharding: nodes partitioned by dst ownership (nsh=12500/core). Per layer each
core computes hw_aug rows (128 hw feats | a_src-dot | pad, bf16, 264B) for
ITS shard only, then an AllGather replicates the 13MB table. Edges are
dst-sorted into 128-edge tiles covering K consecutive dst nodes (K chosen to
minimize gather descriptors; groups capped at 128 edges). Each tile's rows
are fetched by one indirect DMA (128 row descriptors; the gather is
HBM-latency-bound at ~37ns/row, so bytes are nearly free). Edge math
(leaky-relu/exp/slot-select) runs in bf16; per tile two bf16 PE matmuls
produce K x 129 segment sums (features + softmax denominator) into an outbuf
whose row index IS the local dst index; outbuf writes ride the scalar-engine
HWDGE queue so index/slot loads on SP are never queued behind them. Edges
overflowing a K-group (~1%) go through a spill pass: indirect gathers of row
and ad values, a 128-slot one-hot matmul, and an indirect scatter-ADD (CCE)
into the same outbuf. Finalize divides by the denominator, applies
graph-LayerNorm via AllReduce of (sum, sumsq), relu, residual. Decoder
computes per-core partial sigmoid-sums; host adds the 8 partials.
"""

import numpy as np
import ml_dtypes

NC = 8
P = 128
D = 128
DW = 132          # bf16 row: 128 feats + as + 3 pad (264B)
L = 3
EPS = 1e-5
NEG = 0.2
CH = 256          # pre-pass node chunk (a=2 of 128)
G = 64            # edge tiles per loop group
NSH = 12500
SH_PAD = 12544    # shard rows padded to 98*128

_CACHE = {}
SP_MODE = False


def _pack_spill(spill, t_spad, t_pad_k):
    """spill: list of (dstloc, src_row) grouped by dst. Pack into tiles of
    <=P edges, each distinct dst -> one slot, dst kept within one tile."""
    srcS = np.zeros((t_spad, P), np.int32)
    slotS = np.full((t_spad, P), float(P), np.float32)   # sentinel P
    dstS = np.zeros((t_spad, P), np.int32)
    rixS = np.empty((t_spad, P), np.int32)
    for t in range(t_spad):
        rixS[t] = t_pad_k + np.arange(P)                 # trash rows
    t = 0
    epos = 0
    slot = 0
    i = 0
    while i < len(spill):
        d = spill[i][0]
        j = i
        while j < len(spill) and spill[j][0] == d:
            j += 1
        bunch = j - i
        if epos + bunch > P or slot >= P:
            t += 1
            epos = 0
            slot = 0
        for dd, sr in spill[i:j]:
            srcS[t, epos] = sr
            slotS[t, epos] = float(slot)
            dstS[t, epos] = dd
            epos += 1
        rixS[t, slot] = d
        slot += 1
        i = j
    assert t < t_spad or (t == t_spad and epos == 0) or t_spad == 0
    return srcS, slotS, dstS, rixS


def _prep(src, dst, n_full, nsh):
    perm = np.argsort(dst, kind="stable")
    src_s = np.ascontiguousarray(src[perm]).astype(np.int64)
    dst_s = np.ascontiguousarray(dst[perm]).astype(np.int64)
    starts = np.searchsorted(dst_s, np.arange(n_full + 1, dtype=np.int64), "left")
    shard_of = src_s // NSH
    row_of = (shard_of * SH_PAD + (src_s - shard_of * NSH)).astype(np.int64)

    counts_all = np.zeros((NC, nsh), np.int64)
    datas = []
    for c in range(NC):
        lo = c * nsh
        e0, e1 = starts[lo], starts[lo + nsh]
        dl = dst_s[e0:e1] - lo
        counts_all[c] = np.bincount(dl, minlength=nsh)
        datas.append((dl, row_of[e0:e1]))

    # choose k minimizing gather descriptors: main slots + 3*128 per spill tile
    best = None
    for k in range(4, 10):
        ngrp = (nsh + k - 1) // k
        t_pad = ((ngrp + G - 1) // G) * G
        worst_tiles = 0
        for c in range(NC):
            gs = np.add.reduceat(counts_all[c], np.arange(0, nsh, k))
            se = int(np.maximum(gs - P, 0).sum())
            worst_tiles = max(worst_tiles, (se + 99) // 100)
        cost = t_pad * P + worst_tiles * 3 * P
        if best is None or cost < best[0]:
            best = (cost, k, t_pad, worst_tiles)
    _, k, t_pad, t_spad = best
    t_spad = t_spad + 1 if t_spad else 0   # slack tile

    ngrp = (nsh + k - 1) // k
    per = []
    for c in range(NC):
        dl, iq = datas[c]
        g = dl // k
        s = (dl % k).astype(np.float32)
        order = np.argsort(g, kind="stable")
        g, s, iq = g[order], s[order], iq[order]
        grp_first = np.searchsorted(g, np.arange(ngrp))
        inpos = np.arange(len(g)) - grp_first[g]
        keep = inpos < P
        epos = g[keep] * P + inpos[keep]
        srcA = np.zeros(t_pad * P, np.int32)
        slotA = np.full(t_pad * P, float(k), np.float32)
        srcA[epos] = iq[keep].astype(np.int32)
        slotA[epos] = s[keep]
        sp_dl = (g[~keep] * k + s[~keep].astype(np.int64)).astype(np.int64)
        spill = sorted(zip(sp_dl.tolist(), iq[~keep].tolist()))
        srcS, slotS, dstS, rixS = _pack_spill(spill, t_spad, t_pad * k)
        per.append({
            "srcA": np.ascontiguousarray(srcA.reshape(t_pad, P).T),
            "slotW": np.ascontiguousarray(
                slotA.reshape(t_pad, P).T).astype(ml_dtypes.bfloat16),
            "srcS": np.ascontiguousarray(srcS.T),
            "slotS": np.ascontiguousarray(slotS.T).astype(ml_dtypes.bfloat16),
            "dstS": np.ascontiguousarray(dstS.T),
            "rixS": np.ascontiguousarray(rixS.T),
        })
    return per, (k, t_pad, t_spad)


def _build(nsh, fr, k, t_pad, t_spad, ncores):
    import concourse.bacc as bacc
    import concourse.tile as tile
    from concourse import mybir
    from concourse.bass import IndirectOffsetOnAxis, ds
    from concourse.masks import make_identity

    f32 = mybir.dt.float32
    bf16 = mybir.dt.bfloat16
    i16 = mybir.dt.int16
    AT = mybir.ActivationFunctionType
    OP = mybir.AluOpType

    n_tab = ncores * SH_PAD
    gk = G * k
    nd_inv = 1.0 / (float(nsh * ncores) * D)
    ad_pad = max(t_pad * k + P, SH_PAD)

    nc = bacc.Bacc()
    xs = nc.declare_dram_parameter("xs", [nsh, D], f32, isOutput=False)
    encW = nc.declare_dram_parameter("encW", [D, D], f32, isOutput=False)
    encb = nc.declare_dram_parameter("encb", [P, D], f32, isOutput=False)
    WgP = nc.declare_dram_parameter("WgP", [L, D, D], f32, isOutput=False)
    a2P = nc.declare_dram_parameter("a2P", [L, D, 2], f32, isOutput=False)
    bgP = nc.declare_dram_parameter("bgP", [L, P, D], f32, isOutput=False)
    lnwP = nc.declare_dram_parameter("lnwP", [L, P, D], f32, isOutput=False)
    lnbP = nc.declare_dram_parameter("lnbP", [L, P, D], f32, isOutput=False)
    decW = nc.declare_dram_parameter("decW", [D, 1], f32, isOutput=False)
    decb = nc.declare_dram_parameter("decb", [1, 1], f32, isOutput=False)
    srcA = nc.declare_dram_parameter("srcA", [P, t_pad], mybir.dt.int32,
                                     isOutput=False)
    slotWp = nc.declare_dram_parameter("slotW", [P, t_pad], bf16, isOutput=False)
    iotaP = nc.declare_dram_parameter("iotaP", [P, gk], bf16, isOutput=False)
    if t_spad:
        srcSp = nc.declare_dram_parameter("srcS", [P, t_spad], mybir.dt.int32,
                                          isOutput=False)
        slotSp = nc.declare_dram_parameter("slotS", [P, t_spad], bf16,
                                           isOutput=False)
        dstSp = nc.declare_dram_parameter("dstS", [P, t_spad], mybir.dt.int32,
                                          isOutput=False)
        rixSp = nc.declare_dram_parameter("rixS", [P, t_spad], mybir.dt.int32,
                                          isOutput=False)
        iotaSP = nc.declare_dram_parameter("iotaS", [P, P], bf16, isOutput=False)
    outp = nc.declare_dram_parameter("outp", [1, 1], f32, isOutput=True)

    cc_bf = nc.dram_tensor("cc_bf", [SH_PAD, DW], bf16)
    hw_aug = nc.dram_tensor("hw_aug", [n_tab, DW], bf16, addr_space="Shared")
    st_in = nc.dram_tensor("st_in", [1, 2], f32)
    st_out = nc.dram_tensor("st_out", [1, 2], f32, addr_space="Shared")
    ad_dram = nc.dram_tensor("ad_dram", [1, ad_pad], f32)
    ad2 = nc.dram_tensor("ad2", [ad_pad, 1], f32)
    outbuf = nc.dram_tensor("outbuf", [t_pad * k + P, D + 1], f32)
    conv_dram = nc.dram_tensor("conv_dram", [nsh, D], f32)
    hsh_dram = nc.dram_tensor("hsh_dram", [SH_PAD, D], f32)
    rg = [list(range(ncores))]

    with tile.TileContext(nc) as tc:
        with (
            tc.tile_pool(name="cst", bufs=1) as cst,
            tc.tile_pool(name="wts", bufs=1) as wts,
            tc.tile_pool(name="sml", bufs=2) as sml,
        ):
            ident = cst.tile([P, P], f32)
            make_identity(nc, ident[:])
            ones_col = cst.tile([P, 1], bf16)
            nc.vector.memset(ones_col[:], 1.0)
            one_row = cst.tile([1, P], bf16)
            nc.vector.memset(one_row[:], 1.0)
            one_row_f = cst.tile([1, P], f32)
            nc.vector.memset(one_row_f[:], 1.0)
            ones_col_f = cst.tile([P, 1], f32)
            nc.vector.memset(ones_col_f[:], 1.0)
            iotaB = cst.tile([P, gk], bf16)
            nc.sync.dma_start(iotaB[:], iotaP[:])
            if t_spad:
                iotaS = cst.tile([P, P], bf16)
                nc.sync.dma_start(iotaS[:], iotaSP[:])
            # zero hsh pad rows once
            zrow = cst.tile([P, D], f32)
            nc.vector.memset(zrow[:], 0.0)
            if SH_PAD > nsh:
                nc.sync.dma_start(hsh_dram[nsh:SH_PAD, :], zrow[:SH_PAD - nsh, :])

            encW_sb = wts.tile([D, D], f32)
            nc.sync.dma_start(encW_sb[:], encW[:])
            encb_sb = wts.tile([P, D], f32)
            nc.sync.dma_start(encb_sb[:], encb[:])
            decW_sb = wts.tile([D, 1], f32)
            nc.sync.dma_start(decW_sb[:], decW[:])
            decb_sb = wts.tile([1, 1], f32)
            nc.sync.dma_start(decb_sb[:], decb[:])

            # ---------------- encoder ----------------
            with (
                tc.tile_pool(name="ep", bufs=3) as ep,
                tc.tile_pool(name="eps", bufs=2, space="PSUM") as eps,
            ):
                def enc_body(iv):
                    xt = ep.tile([fr, D], f32, tag="xt")
                    nc.sync.dma_start(xt[:], xs[ds(iv, fr), :])
                    pT = eps.tile([D, fr], f32, tag="pT")
                    nc.tensor.transpose(pT[:], xt[:], ident[:fr, :fr])
                    xT = ep.tile([D, fr], f32, tag="xT")
                    nc.vector.tensor_copy(xT[:], pT[:])
                    ph = eps.tile([fr, D], f32, tag="ph")
                    nc.tensor.matmul(ph[:], xT[:], encW_sb[:], start=True, stop=True)
                    h0 = ep.tile([fr, D], f32, tag="h0")
                    nc.vector.tensor_tensor(out=h0[:], in0=ph[:], in1=encb_sb[:fr, :], op=OP.add)
                    nc.sync.dma_start(hsh_dram[ds(iv, fr), :], h0[:])
                tc.For_i_unrolled(0, nsh, fr, enc_body, max_unroll=4)

            # ---------------- layers ----------------
            for l in range(L):
                with tc.tile_pool(name=f"lw{l}", bufs=1) as lw:
                    Wg_sb = lw.tile([D, D], f32)
                    nc.sync.dma_start(Wg_sb[:], WgP[l, :, :])
                    a2_sb = lw.tile([D, 2], f32)
                    nc.sync.dma_start(a2_sb[:], a2P[l, :, :])
                    bg_sb = lw.tile([P, D], f32)
                    nc.sync.dma_start(bg_sb[:], bgP[l, :, :])
                    lnw_sb = lw.tile([P, D], f32)
                    nc.sync.dma_start(lnw_sb[:], lnwP[l, :, :])
                    lnb_sb = lw.tile([P, D], f32)
                    nc.sync.dma_start(lnb_sb[:], lnbP[l, :, :])

                    # ---- pre-pass: own-shard hw_aug rows + ad table ----
                    with (
                        tc.tile_pool(name="pp", bufs=3) as pp,
                        tc.tile_pool(name="ppsA", bufs=2, space="PSUM") as ppsA,
                        tc.tile_pool(name="ppsB", bufs=2, space="PSUM") as ppsB,
                        tc.tile_pool(name="ppsC", bufs=1, space="PSUM") as ppsC,
                    ):
                        na = CH // P

                        def pre_body(iv):
                            hch = pp.tile([P, na, D], f32, tag="hch")
                            nc.sync.dma_start(
                                hch[:], hsh_dram[ds(iv, CH), :].rearrange(
                                    "(a p) d -> p a d", p=P))
                            pT = ppsA.tile([P, CH], f32, tag="pT")
                            for b in range(na):
                                nc.tensor.transpose(
                                    pT[:, b * P:(b + 1) * P], hch[:, b, :], ident[:])
                            hT = pp.tile([P, CH], f32, tag="hT")
                            nc.vector.tensor_copy(hT[:], pT[:])
                            phw = ppsB.tile([P, CH], f32, tag="phw")
                            nc.tensor.matmul(phw[:], Wg_sb[:], hT[:], start=True, stop=True)
                            hwT = pp.tile([P, CH], f32, tag="hwT")
                            nc.vector.tensor_copy(hwT[:], phw[:])
                            pas = ppsC.tile([2, CH], f32, tag="pas")
                            nc.tensor.matmul(pas[:], a2_sb[:], hwT[:], start=True, stop=True)
                            phw2 = ppsA.tile([P, CH], f32, tag="phw2")
                            for b in range(na):
                                nc.tensor.transpose(
                                    phw2[:, b * P:(b + 1) * P],
                                    hwT[:, b * P:(b + 1) * P], ident[:])
                            asv = pp.tile([2, CH], f32, tag="asv")
                            nc.vector.tensor_copy(asv[:], pas[:])
                            pasT = ppsC.tile([P, 2 * na], f32, tag="pasT")
                            for b in range(na):
                                nc.tensor.transpose(
                                    pasT[:, b * 2:(b + 1) * 2],
                                    asv[:, b * P:(b + 1) * P], ident[:2, :2])
                            stg = pp.tile([P, na, DW], bf16, tag="stg")
                            nc.vector.tensor_copy(
                                stg[:, :, 0:D], phw2[:].rearrange("p (a d) -> p a d", a=na))
                            nc.scalar.activation(
                                stg[:, :, D:D + 1],
                                pasT[:].rearrange("p (a t) -> p a t", a=na)[:, :, 0:1],
                                AT.Copy)
                            nc.vector.memset(stg[:, :, D + 1:DW], 0.0)
                            nc.sync.dma_start(
                                cc_bf[ds(iv, CH), :].rearrange("(a p) d -> p a d", p=P),
                                stg[:])
                            nc.sync.dma_start(ad_dram[0:1, ds(iv, CH)], asv[1:2, :])
                            if t_spad:
                                nc.sync.dma_start(
                                    ad2[ds(iv, CH), :].rearrange("n d -> d n"),
                                    asv[1:2, :])
                        tc.For_i_unrolled(0, SH_PAD, CH, pre_body, max_unroll=4)
                        if ad_pad > SH_PAD:
                            zr = pp.tile([1, ad_pad - SH_PAD], f32)
                            nc.vector.memset(zr[:], 0.0)
                            nc.sync.dma_start(ad_dram[0:1, SH_PAD:ad_pad], zr[:])

                    nc.gpsimd.collective_compute(
                        "AllGather", OP.bypass, replica_groups=rg,
                        ins=[cc_bf[:, :]], outs=[hw_aug[:, :]])

                    # ---- edge loop ----
                    with (
                        tc.tile_pool(name="eb", bufs=6) as eb,
                        tc.tile_pool(name="ebR", bufs=2, space="PSUM") as ebR,
                        tc.tile_pool(name="ebo", bufs=6, space="PSUM") as ebo,
                    ):
                        for i0 in range(0, t_pad, G):
                            adr = eb.tile([1, gk], f32, tag="adr")
                            nc.sync.dma_start(
                                adr[:], ad_dram[0:1, ds(i0 * k, gk)])
                            adb = eb.tile([1, gk], bf16, tag="adb")
                            nc.vector.tensor_copy(adb[:], adr[:])
                            Rp = ebR.tile([P, gk], f32, tag="Rp")
                            nc.tensor.matmul(Rp[:], one_row[:], adb[:, 0:gk],
                                             start=True, stop=True)
                            Rb = eb.tile([P, gk], bf16, tag="Rb")
                            nc.vector.tensor_copy(Rb[:], Rp[:])
                            six = eb.tile([P, G], mybir.dt.int32, tag="six")
                            nc.sync.dma_start(six[:], srcA[:, ds(i0, G)])
                            slb = eb.tile([P, G, 1], bf16, tag="slb")
                            nc.sync.dma_start(slb[:, :, 0], slotWp[:, ds(i0, G)])
                            msgb = eb.tile([P, G, DW], bf16, tag="msgb")
                            for j in range(G):
                                nc.gpsimd.indirect_dma_start(
                                    out=msgb[:, j, :], out_offset=None, in_=hw_aug[:],
                                    in_offset=IndirectOffsetOnAxis(
                                        ap=six[:, j:j + 1], axis=0))
                            Bt = eb.tile([P, G, k], bf16, tag="Bt")
                            nc.vector.tensor_tensor(
                                out=Bt[:],
                                in0=Rb[:].rearrange("p (g s) -> p g s", g=G),
                                in1=msgb[:, :, D:D + 1].to_broadcast([P, G, k]),
                                op=OP.add)
                            lr = eb.tile([P, G, k], bf16, tag="lr")
                            nc.vector.tensor_scalar_mul(lr[:], Bt[:], NEG)
                            nc.vector.tensor_tensor(out=lr[:], in0=lr[:], in1=Bt[:], op=OP.max)
                            M0 = eb.tile([P, G, k], bf16, tag="M0")
                            nc.scalar.activation(M0[:], lr[:], AT.Exp)
                            Sel = eb.tile([P, G, k], bf16, tag="Sel")
                            nc.vector.tensor_tensor(
                                out=Sel[:],
                                in0=slb[:].to_broadcast([P, G, k]),
                                in1=iotaB[:].rearrange("p (g s) -> p g s", g=G),
                                op=OP.is_equal)
                            nc.vector.tensor_tensor(out=Sel[:], in0=Sel[:], in1=M0[:], op=OP.mult)
                            for j in range(G):
                                po = ebo.tile([k, D + 1], f32, tag="po")
                                nc.tensor.matmul(po[:, 0:D], Sel[:, j, :],
                                                 msgb[:, j, 0:D], start=True, stop=True)
                                nc.tensor.matmul(po[:, D:D + 1], Sel[:, j, :],
                                                 ones_col[:], start=True, stop=True)
                                st = eb.tile([k, D + 1], f32, tag="st")
                                if j % 2 == 0:
                                    nc.vector.tensor_copy(st[:], po[:])
                                else:
                                    nc.scalar.activation(st[:], po[:], AT.Copy)
                                nc.sync.dma_start(
                                    outbuf[ds(i0 * k + j * k, k), :], st[:])

                    # ---- spill pass: overflow edges, indirect scatter-add ----
                    if t_spad:
                        with (
                            tc.tile_pool(name="sp", bufs=3) as spp,
                            tc.tile_pool(name="spo", bufs=2, space="PSUM") as spo,
                        ):
                            with tc.For_i(0, t_spad, 1) as s0:
                                sxt = spp.tile([P, 1], mybir.dt.int32, tag="sxt")
                                nc.sync.dma_start(sxt[:], srcSp[:, ds(s0, 1)])
                                dxt = spp.tile([P, 1], mybir.dt.int32, tag="dxt")
                                nc.sync.dma_start(dxt[:], dstSp[:, ds(s0, 1)])
                                rxt = spp.tile([P, 1], mybir.dt.int32, tag="rxt")
                                nc.sync.dma_start(rxt[:], rixSp[:, ds(s0, 1)])
                                slb1 = spp.tile([P, 1], bf16, tag="slb1")
                                nc.sync.dma_start(slb1[:], slotSp[:, ds(s0, 1)])
                                msg1 = spp.tile([P, DW], bf16, tag="msg1")
                                nc.gpsimd.indirect_dma_start(
                                    out=msg1[:], out_offset=None, in_=hw_aug[:],
                                    in_offset=IndirectOffsetOnAxis(
                                        ap=sxt[:, 0:1], axis=0))
                                adc = spp.tile([P, 1], f32, tag="adc")
                                nc.gpsimd.indirect_dma_start(
                                    out=adc[:], out_offset=None, in_=ad2[:],
                                    in_offset=IndirectOffsetOnAxis(
                                        ap=dxt[:, 0:1], axis=0))
                                Bt1 = spp.tile([P, 1], bf16, tag="Bt1")
                                nc.vector.tensor_copy(Bt1[:], adc[:])
                                nc.vector.tensor_tensor(
                                    out=Bt1[:], in0=Bt1[:],
                                    in1=msg1[:, D:D + 1], op=OP.add)
                                lr1 = spp.tile([P, 1], bf16, tag="lr1")
                                nc.vector.tensor_scalar_mul(lr1[:], Bt1[:], NEG)
                                nc.vector.tensor_tensor(
                                    out=lr1[:], in0=lr1[:], in1=Bt1[:], op=OP.max)
                                M1 = spp.tile([P, 1], bf16, tag="M1")
                                nc.scalar.activation(M1[:], lr1[:], AT.Exp)
                                Sel1 = spp.tile([P, P], bf16, tag="Sel1")
                                nc.vector.tensor_tensor(
                                    out=Sel1[:],
                                    in0=slb1[:].to_broadcast([P, P]),
                                    in1=iotaS[:], op=OP.is_equal)
                                nc.vector.tensor_tensor(
                                    out=Sel1[:], in0=Sel1[:],
                                    in1=M1[:].to_broadcast([P, P]), op=OP.mult)
                                po1 = spo.tile([P, D + 1], f32, tag="po1")
                                nc.tensor.matmul(po1[:, 0:D], Sel1[:],
                                                 msgb_sp := msg1[:, 0:D],
                                                 start=True, stop=True)
                                nc.tensor.matmul(po1[:, D:D + 1], Sel1[:],
                                                 ones_col[:], start=True, stop=True)
                                st1 = spp.tile([P, D + 1], f32, tag="st1")
                                nc.vector.tensor_copy(st1[:], po1[:])
                                nc.gpsimd.indirect_dma_start(
                                    out=outbuf[:], out_offset=IndirectOffsetOnAxis(
                                        ap=rxt[:, 0:1], axis=0),
                                    in_=st1[:], in_offset=None,
                                    compute_op=OP.add)

                    # ---- finalize: conv = (sum of 4 outbufs)/denom + bg ----
                    with (
                        tc.tile_pool(name="fp", bufs=3) as fp,
                        tc.tile_pool(name="facc", bufs=1) as facc,
                        tc.tile_pool(name="fps", bufs=2, space="PSUM") as fps,
                    ):
                        acc = facc.tile([fr, 2], f32)
                        nc.vector.memset(acc[:], 0.0)

                        def fin_body(iv):
                            ob = fp.tile([fr, D + 1], f32, tag="ob")
                            nc.sync.dma_start(ob[:], outbuf[ds(iv, fr), :])
                            rcp = fp.tile([fr, 1], f32, tag="rcp")
                            nc.vector.reciprocal(rcp[:], ob[:, D:D + 1])
                            cv = fp.tile([fr, D], f32, tag="cv")
                            nc.vector.tensor_tensor(
                                out=cv[:], in0=ob[:, 0:D],
                                in1=rcp[:].to_broadcast([fr, D]), op=OP.mult)
                            nc.vector.tensor_tensor(
                                out=cv[:], in0=cv[:], in1=bg_sb[:fr, :], op=OP.add)
                            s1 = fp.tile([fr, 1], f32, tag="s1")
                            nc.vector.tensor_reduce(
                                out=s1[:], in_=cv[:], axis=mybir.AxisListType.X, op=OP.add)
                            sqv = fp.tile([fr, D], f32, tag="sqv")
                            s2 = fp.tile([fr, 1], f32, tag="s2")
                            nc.scalar.activation(sqv[:], cv[:], AT.Square, accum_out=s2[:])
                            nc.vector.tensor_tensor(
                                out=acc[:, 0:1], in0=acc[:, 0:1], in1=s1[:], op=OP.add)
                            nc.vector.tensor_tensor(
                                out=acc[:, 1:2], in0=acc[:, 1:2], in1=s2[:], op=OP.add)
                            nc.sync.dma_start(conv_dram[ds(iv, fr), :], cv[:])
                        tc.For_i_unrolled(0, nsh, fr, fin_body, max_unroll=4)

                        pst = fps.tile([1, 2], f32)
                        nc.tensor.matmul(pst[:], ones_col_f[:fr, :], acc[:], start=True, stop=True)
                        stt = sml.tile([1, 2], f32, tag="stt")
                        nc.vector.tensor_copy(stt[:], pst[:])
                        nc.sync.dma_start(st_in[:, :], stt[:])

                    nc.gpsimd.collective_compute(
                        "AllReduce", OP.add, replica_groups=rg,
                        ins=[st_in[:, :]], outs=[st_out[:, :]])

                    # ---- stats -> scale/shift, apply LN + relu + residual ----
                    with (
                        tc.tile_pool(name="ap", bufs=3) as apl,
                        tc.tile_pool(name="aps", bufs=2, space="PSUM") as aps,
                    ):
                        sto = sml.tile([1, 2], f32, tag="sto")
                        nc.sync.dma_start(sto[:], st_out[:, :])
                        mn = sml.tile([1, 1], f32, tag="mn")
                        nc.vector.tensor_scalar_mul(mn[:], sto[:, 0:1], nd_inv)
                        ms = sml.tile([1, 1], f32, tag="ms")
                        nc.vector.tensor_scalar_mul(ms[:], sto[:, 1:2], nd_inv)
                        m2 = sml.tile([1, 1], f32, tag="m2")
                        nc.vector.tensor_tensor(out=m2[:], in0=mn[:], in1=mn[:], op=OP.mult)
                        vr = sml.tile([1, 1], f32, tag="vr")
                        nc.vector.tensor_tensor(out=vr[:], in0=ms[:], in1=m2[:], op=OP.subtract)
                        nc.vector.tensor_scalar_add(vr[:], vr[:], EPS)
                        sd = sml.tile([1, 1], f32, tag="sd")
                        nc.scalar.activation(sd[:], vr[:], AT.Sqrt)
                        rs = sml.tile([1, 1], f32, tag="rs")
                        nc.vector.reciprocal(rs[:], sd[:])
                        nmr = sml.tile([1, 1], f32, tag="nmr")
                        nc.vector.tensor_tensor(out=nmr[:], in0=mn[:], in1=rs[:], op=OP.mult)
                        nc.vector.tensor_scalar_mul(nmr[:], nmr[:], -1.0)
                        pk = sml.tile([1, 2], f32, tag="pk")
                        nc.vector.tensor_copy(pk[:, 0:1], rs[:])
                        nc.vector.tensor_copy(pk[:, 1:2], nmr[:])
                        pbc = aps.tile([P, 2], f32)
                        nc.tensor.matmul(pbc[:], one_row_f[:], pk[:], start=True, stop=True)
                        bc = sml.tile([P, 2], f32, tag="bc")
                        nc.vector.tensor_copy(bc[:], pbc[:])

                        def app_body(iv):
                            cv = apl.tile([fr, D], f32, tag="acv")
                            nc.sync.dma_start(cv[:], conv_dram[ds(iv, fr), :])
                            tt = apl.tile([fr, D], f32, tag="att")
                            nc.vector.tensor_scalar(
                                out=tt[:], in0=cv[:], scalar1=bc[:fr, 0:1],
                                scalar2=bc[:fr, 1:2], op0=OP.mult, op1=OP.add)
                            nc.vector.tensor_tensor(
                                out=tt[:], in0=tt[:], in1=lnw_sb[:fr, :], op=OP.mult)
                            nc.vector.tensor_tensor(
                                out=tt[:], in0=tt[:], in1=lnb_sb[:fr, :], op=OP.add)
                            nc.vector.tensor_scalar_max(tt[:], tt[:], 0.0)
                            hin = apl.tile([fr, D], f32, tag="hin")
                            nc.sync.dma_start(hin[:], hsh_dram[ds(iv, fr), :])
                            nc.vector.tensor_tensor(
                                out=tt[:], in0=tt[:], in1=hin[:], op=OP.add)
                            nc.sync.dma_start(hsh_dram[ds(iv, fr), :], tt[:])
                        tc.For_i_unrolled(0, nsh, fr, app_body, max_unroll=4)

            # ---------------- decoder ----------------
            with (
                tc.tile_pool(name="dp", bufs=3) as dp,
                tc.tile_pool(name="dacc", bufs=1) as dac,
                tc.tile_pool(name="dps", bufs=2, space="PSUM") as dps,
            ):
                dacc = dac.tile([1, 1], f32)
                nc.vector.memset(dacc[:], 0.0)

                def dec_body(iv):
                    ch = dp.tile([fr, D], f32, tag="ch")
                    nc.sync.dma_start(ch[:], hsh_dram[ds(iv, fr), :])
                    pT = dps.tile([D, fr], f32, tag="dpT")
                    nc.tensor.transpose(pT[:], ch[:], ident[:fr, :fr])
                    hT = dp.tile([D, fr], f32, tag="hT")
                    nc.vector.tensor_copy(hT[:], pT[:])
                    pz = dps.tile([1, fr], f32, tag="pz")
                    nc.tensor.matmul(pz[:], decW_sb[:], hT[:], start=True, stop=True)
                    zz = dp.tile([1, fr], f32, tag="zz")
                    zs = dp.tile([1, 1], f32, tag="zs")
                    nc.scalar.activation(zz[:], pz[:], AT.Sigmoid,
                                         bias=decb_sb[:], accum_out=zs[:])
                    nc.vector.tensor_tensor(out=dacc[:], in0=dacc[:], in1=zs[:], op=OP.add)
                tc.For_i_unrolled(0, nsh, fr, dec_body, max_unroll=4)
                nc.sync.dma_start(outp[:, :], dacc[:])

    nc.finalize()
    return nc



def _get_nc(nsh, fr, k, t_pad, t_spad, ncores):
    key = (nsh, fr, k, t_pad, t_spad, ncores)
    if key not in _CACHE:
        _CACHE[key] = _build(nsh, fr, k, t_pad, t_spad, ncores)
    return _CACHE[key]


def _prepare(x, edge_index, enc_W, enc_b, Wg, a_src, a_dst, bg, ln_w, ln_b,
             dec_W, dec_b):
    x = np.asarray(x, np.float32)
    n_full = x.shape[0]
    nsh = n_full // NC
    fr = next(f for f in range(min(P, nsh), 0, -1) if nsh % f == 0)
    ei = np.asarray(edge_index)
    loop = np.arange(n_full, dtype=ei.dtype)
    src = np.concatenate([ei[0], loop])
    dst = np.concatenate([ei[1], loop])
    cores, (k, t_pad, t_spad) = _prep(src, dst, n_full, nsh)

    enc_b = np.asarray(enc_b, np.float32)
    Wg = np.asarray(Wg, np.float32)
    a2 = np.stack([np.asarray(a_src, np.float32),
                   np.asarray(a_dst, np.float32)], axis=2)
    bg_r = np.broadcast_to(np.asarray(bg, np.float32)[:, None, :], (L, P, D)).copy()
    lnw_r = np.broadcast_to(np.asarray(ln_w, np.float32)[:, None, :], (L, P, D)).copy()
    lnb_r = np.broadcast_to(np.asarray(ln_b, np.float32)[:, None, :], (L, P, D)).copy()
    encb_r = np.broadcast_to(enc_b[None, :], (P, D)).copy()
    decW_h = np.asarray(dec_W, np.float32).reshape(D, 1)
    decb_h = np.asarray(dec_b, np.float32).reshape(1, 1)

    nc = _get_nc(nsh, fr, k, t_pad, t_spad, NC)
    in_maps = []
    for c in range(NC):
        m = {
            "xs": np.ascontiguousarray(x[c * nsh:(c + 1) * nsh]),
            "encW": np.ascontiguousarray(np.asarray(enc_W, np.float32)),
            "encb": encb_r, "WgP": Wg, "a2P": a2, "bgP": bg_r,
            "lnwP": lnw_r, "lnbP": lnb_r, "decW": decW_h, "decb": decb_h,
        }
        m["srcA"] = cores[c]["srcA"]
        m["slotW"] = cores[c]["slotW"]
        m["iotaP"] = np.broadcast_to(
            np.tile(np.arange(k, dtype=np.float32), G)[None, :],
            (P, G * k)).astype(ml_dtypes.bfloat16).copy()
        if t_spad:
            m["srcS"] = cores[c]["srcS"]
            m["slotS"] = cores[c]["slotS"]
            m["dstS"] = cores[c]["dstS"]
            m["rixS"] = cores[c]["rixS"]
            m["iotaS"] = np.broadcast_to(
                np.arange(P, dtype=np.float32)[None, :],
                (P, P)).astype(ml_dtypes.bfloat16).copy()
        in_maps.append(m)
    return nc, in_maps


def kernel(x, edge_index, enc_W, enc_b, Wg, a_src, a_dst, bg, ln_w, ln_b,
           dec_W, dec_b):
    from concourse.bass_utils import run_bass_kernel_spmd

    nc, in_maps = _prepare(x, edge_index, enc_W, enc_b, Wg, a_src, a_dst,
                           bg, ln_w, ln_b, dec_W, dec_b)
    res = run_bass_kernel_spmd(nc, in_maps, list(range(NC))).results
    total = np.float32(sum(float(res[c]["outp"][0, 0]) for c in range(NC)))
    return np.array([total], np.float32)


# revision 40
# speedup vs baseline: 1.0302x; 1.0302x over previous
"""3-layer GAT on Trainium2, 8 NeuronCores, full computation on device.

Sharding: nodes partitioned by dst ownership (nsh=12500/core). Per layer each
core computes hw_aug rows (128 hw feats | a_src-dot | pad, bf16, 264B) for
ITS shard only, then an AllGather replicates the 13MB table. Edges are
dst-sorted into 128-edge tiles covering K consecutive dst nodes (K chosen to
minimize gather descriptors; groups capped at 128 edges). Each tile's rows
are fetched by one indirect DMA (128 row descriptors; the gather is
HBM-latency-bound at ~37ns/row, so bytes are nearly free). Edge math
(leaky-relu/exp/slot-select) runs in bf16; per tile two bf16 PE matmuls
produce K x 129 segment sums (features + softmax denominator) into an outbuf
whose row index IS the local dst index; outbuf writes ride the scalar-engine
HWDGE queue so index/slot loads on SP are never queued behind them. Edges
overflowing a K-group (~1%) go through a spill pass: indirect gathers of row
and ad values, a 128-slot one-hot matmul, and an indirect scatter-ADD (CCE)
into the same outbuf. Finalize divides by the denominator, applies
graph-LayerNorm via AllReduce of (sum, sumsq), relu, residual. Decoder
computes per-core partial sigmoid-sums; host adds the 8 partials.
"""

import numpy as np
import ml_dtypes

NC = 8
P = 128
D = 128
DW = 132          # bf16 row: 128 feats + as + 3 pad (264B)
L = 3
EPS = 1e-5
NEG = 0.2
CH = 256          # pre-pass node chunk (a=2 of 128)
G = 64            # edge tiles per loop group
NSH = 12500
SH_PAD = 12544    # shard rows padded to 98*128

_CACHE = {}
SP_MODE = False


def _pack_spill(spill, t_spad, t_pad_k):
    """spill: list of (dstloc, src_row) grouped by dst. Pack into tiles of
    <=P edges, each distinct dst -> one slot, dst kept within one tile."""
    srcS = np.zeros((t_spad, P), np.int32)
    slotS = np.full((t_spad, P), float(P), np.float32)   # sentinel P
    dstS = np.zeros((t_spad, P), np.int32)
    rixS = np.empty((t_spad, P), np.int32)
    for t in range(t_spad):
        rixS[t] = t_pad_k + np.arange(P)                 # trash rows
    t = 0
    epos = 0
    slot = 0
    i = 0
    while i < len(spill):
        d = spill[i][0]
        j = i
        while j < len(spill) and spill[j][0] == d:
            j += 1
        bunch = j - i
        if epos + bunch > P or slot >= P:
            t += 1
            epos = 0
            slot = 0
        for dd, sr in spill[i:j]:
            srcS[t, epos] = sr
            slotS[t, epos] = float(slot)
            dstS[t, epos] = dd
            epos += 1
        rixS[t, slot] = d
        slot += 1
        i = j
    assert t < t_spad or (t == t_spad and epos == 0) or t_spad == 0
    return srcS, slotS, dstS, rixS


def _prep(src, dst, n_full, nsh):
    perm = np.argsort(dst, kind="stable")
    src_s = np.ascontiguousarray(src[perm]).astype(np.int64)
    dst_s = np.ascontiguousarray(dst[perm]).astype(np.int64)
    starts = np.searchsorted(dst_s, np.arange(n_full + 1, dtype=np.int64), "left")
    shard_of = src_s // NSH
    row_of = (shard_of * SH_PAD + (src_s - shard_of * NSH)).astype(np.int64)

    counts_all = np.zeros((NC, nsh), np.int64)
    datas = []
    for c in range(NC):
        lo = c * nsh
        e0, e1 = starts[lo], starts[lo + nsh]
        dl = dst_s[e0:e1] - lo
        counts_all[c] = np.bincount(dl, minlength=nsh)
        datas.append((dl, row_of[e0:e1]))

    # choose k minimizing gather descriptors: main slots + 3*128 per spill tile
    best = None
    for k in range(4, 10):
        ngrp = (nsh + k - 1) // k
        t_pad = ((ngrp + G - 1) // G) * G
        worst_tiles = 0
        for c in range(NC):
            gs = np.add.reduceat(counts_all[c], np.arange(0, nsh, k))
            se = int(np.maximum(gs - P, 0).sum())
            worst_tiles = max(worst_tiles, (se + 99) // 100)
        cost = t_pad * P + worst_tiles * 3 * P
        if best is None or cost < best[0]:
            best = (cost, k, t_pad, worst_tiles)
    _, k, t_pad, t_spad = best
    t_spad = t_spad + 1 if t_spad else 0   # slack tile

    ngrp = (nsh + k - 1) // k
    per = []
    for c in range(NC):
        dl, iq = datas[c]
        g = dl // k
        s = (dl % k).astype(np.float32)
        order = np.argsort(g, kind="stable")
        g, s, iq = g[order], s[order], iq[order]
        grp_first = np.searchsorted(g, np.arange(ngrp))
        inpos = np.arange(len(g)) - grp_first[g]
        keep = inpos < P
        epos = g[keep] * P + inpos[keep]
        srcA = np.zeros(t_pad * P, np.int32)
        slotA = np.full(t_pad * P, float(k), np.float32)
        srcA[epos] = iq[keep].astype(np.int32)
        slotA[epos] = s[keep]
        sp_dl = (g[~keep] * k + s[~keep].astype(np.int64)).astype(np.int64)
        spill = sorted(zip(sp_dl.tolist(), iq[~keep].tolist()))
        srcS, slotS, dstS, rixS = _pack_spill(spill, t_spad, t_pad * k)
        per.append({
            "srcA": np.ascontiguousarray(srcA.reshape(t_pad, P).T),
            "slotW": np.ascontiguousarray(
                slotA.reshape(t_pad, P).T).astype(ml_dtypes.bfloat16),
            "srcS": np.ascontiguousarray(srcS.T),
            "slotS": np.ascontiguousarray(slotS.T).astype(ml_dtypes.bfloat16),
            "dstS": np.ascontiguousarray(dstS.T),
            "rixS": np.ascontiguousarray(rixS.T),
        })
    return per, (k, t_pad, t_spad)


def _build(nsh, fr, k, t_pad, t_spad, ncores):
    import concourse.bacc as bacc
    import concourse.tile as tile
    from concourse import mybir
    from concourse.bass import IndirectOffsetOnAxis, ds
    from concourse.masks import make_identity

    f32 = mybir.dt.float32
    bf16 = mybir.dt.bfloat16
    i16 = mybir.dt.int16
    AT = mybir.ActivationFunctionType
    OP = mybir.AluOpType

    n_tab = ncores * SH_PAD
    gk = G * k
    nd_inv = 1.0 / (float(nsh * ncores) * D)
    ad_pad = max(t_pad * k + P, SH_PAD)

    nc = bacc.Bacc()
    xs = nc.declare_dram_parameter("xs", [nsh, D], f32, isOutput=False)
    encW = nc.declare_dram_parameter("encW", [D, D], f32, isOutput=False)
    encb = nc.declare_dram_parameter("encb", [P, D], f32, isOutput=False)
    WgP = nc.declare_dram_parameter("WgP", [L, D, D], f32, isOutput=False)
    a2P = nc.declare_dram_parameter("a2P", [L, D, 2], f32, isOutput=False)
    bgP = nc.declare_dram_parameter("bgP", [L, P, D], f32, isOutput=False)
    lnwP = nc.declare_dram_parameter("lnwP", [L, P, D], f32, isOutput=False)
    lnbP = nc.declare_dram_parameter("lnbP", [L, P, D], f32, isOutput=False)
    decW = nc.declare_dram_parameter("decW", [D, 1], f32, isOutput=False)
    decb = nc.declare_dram_parameter("decb", [1, 1], f32, isOutput=False)
    srcA = nc.declare_dram_parameter("srcA", [P, t_pad], mybir.dt.int32,
                                     isOutput=False)
    slotWp = nc.declare_dram_parameter("slotW", [P, t_pad], bf16, isOutput=False)
    iotaP = nc.declare_dram_parameter("iotaP", [P, gk], bf16, isOutput=False)
    if t_spad:
        srcSp = nc.declare_dram_parameter("srcS", [P, t_spad], mybir.dt.int32,
                                          isOutput=False)
        slotSp = nc.declare_dram_parameter("slotS", [P, t_spad], bf16,
                                           isOutput=False)
        dstSp = nc.declare_dram_parameter("dstS", [P, t_spad], mybir.dt.int32,
                                          isOutput=False)
        rixSp = nc.declare_dram_parameter("rixS", [P, t_spad], mybir.dt.int32,
                                          isOutput=False)
        iotaSP = nc.declare_dram_parameter("iotaS", [P, P], bf16, isOutput=False)
    outp = nc.declare_dram_parameter("outp", [1, 1], f32, isOutput=True)

    cc_bf = nc.dram_tensor("cc_bf", [SH_PAD, DW], bf16)
    hw_aug = nc.dram_tensor("hw_aug", [n_tab, DW], bf16, addr_space="Shared")
    st_in = nc.dram_tensor("st_in", [1, 2], f32)
    st_out = nc.dram_tensor("st_out", [1, 2], f32, addr_space="Shared")
    ad_dram = nc.dram_tensor("ad_dram", [1, ad_pad], f32)
    ad2 = nc.dram_tensor("ad2", [ad_pad, 1], f32)
    outbuf = nc.dram_tensor("outbuf", [t_pad * k + P, D + 1], f32)
    conv_dram = nc.dram_tensor("conv_dram", [nsh, D], f32)
    hsh_dram = nc.dram_tensor("hsh_dram", [SH_PAD, D], f32)
    rg = [list(range(ncores))]

    with tile.TileContext(nc) as tc:
        with (
            tc.tile_pool(name="cst", bufs=1) as cst,
            tc.tile_pool(name="wts", bufs=1) as wts,
            tc.tile_pool(name="sml", bufs=2) as sml,
        ):
            ident = cst.tile([P, P], f32)
            make_identity(nc, ident[:])
            ones_col = cst.tile([P, 1], bf16)
            nc.vector.memset(ones_col[:], 1.0)
            one_row = cst.tile([1, P], bf16)
            nc.vector.memset(one_row[:], 1.0)
            one_row_f = cst.tile([1, P], f32)
            nc.vector.memset(one_row_f[:], 1.0)
            ones_col_f = cst.tile([P, 1], f32)
            nc.vector.memset(ones_col_f[:], 1.0)
            iotaB = cst.tile([P, gk], bf16)
            nc.sync.dma_start(iotaB[:], iotaP[:])
            srcA_sb = cst.tile([P, t_pad], mybir.dt.int32)
            nc.sync.dma_start(srcA_sb[:], srcA[:])
            slotW_sb = cst.tile([P, t_pad], bf16)
            nc.sync.dma_start(slotW_sb[:], slotWp[:])
            if t_spad:
                iotaS = cst.tile([P, P], bf16)
                nc.sync.dma_start(iotaS[:], iotaSP[:])
            # zero hsh pad rows once
            zrow = cst.tile([P, D], f32)
            nc.vector.memset(zrow[:], 0.0)
            if SH_PAD > nsh:
                nc.sync.dma_start(hsh_dram[nsh:SH_PAD, :], zrow[:SH_PAD - nsh, :])

            encW_sb = wts.tile([D, D], f32)
            nc.sync.dma_start(encW_sb[:], encW[:])
            encb_sb = wts.tile([P, D], f32)
            nc.sync.dma_start(encb_sb[:], encb[:])
            decW_sb = wts.tile([D, 1], f32)
            nc.sync.dma_start(decW_sb[:], decW[:])
            decb_sb = wts.tile([1, 1], f32)
            nc.sync.dma_start(decb_sb[:], decb[:])

            # ---------------- encoder ----------------
            with (
                tc.tile_pool(name="ep", bufs=3) as ep,
                tc.tile_pool(name="eps", bufs=2, space="PSUM") as eps,
            ):
                def enc_body(iv):
                    xt = ep.tile([fr, D], f32, tag="xt")
                    nc.sync.dma_start(xt[:], xs[ds(iv, fr), :])
                    pT = eps.tile([D, fr], f32, tag="pT")
                    nc.tensor.transpose(pT[:], xt[:], ident[:fr, :fr])
                    xT = ep.tile([D, fr], f32, tag="xT")
                    nc.vector.tensor_copy(xT[:], pT[:])
                    ph = eps.tile([fr, D], f32, tag="ph")
                    nc.tensor.matmul(ph[:], xT[:], encW_sb[:], start=True, stop=True)
                    h0 = ep.tile([fr, D], f32, tag="h0")
                    nc.vector.tensor_tensor(out=h0[:], in0=ph[:], in1=encb_sb[:fr, :], op=OP.add)
                    nc.sync.dma_start(hsh_dram[ds(iv, fr), :], h0[:])
                tc.For_i_unrolled(0, nsh, fr, enc_body, max_unroll=4)

            # ---------------- layers ----------------
            for l in range(L):
                with tc.tile_pool(name=f"lw{l}", bufs=1) as lw:
                    Wg_sb = lw.tile([D, D], f32)
                    nc.sync.dma_start(Wg_sb[:], WgP[l, :, :])
                    a2_sb = lw.tile([D, 2], f32)
                    nc.sync.dma_start(a2_sb[:], a2P[l, :, :])
                    bg_sb = lw.tile([P, D], f32)
                    nc.sync.dma_start(bg_sb[:], bgP[l, :, :])
                    lnw_sb = lw.tile([P, D], f32)
                    nc.sync.dma_start(lnw_sb[:], lnwP[l, :, :])
                    lnb_sb = lw.tile([P, D], f32)
                    nc.sync.dma_start(lnb_sb[:], lnbP[l, :, :])
                    adb_sb = lw.tile([1, ad_pad], bf16)
                    nc.vector.memset(adb_sb[:, :], 0.0)

                    # ---- pre-pass: own-shard hw_aug rows + ad table ----
                    with (
                        tc.tile_pool(name="pp", bufs=3) as pp,
                        tc.tile_pool(name="ppsA", bufs=2, space="PSUM") as ppsA,
                        tc.tile_pool(name="ppsB", bufs=2, space="PSUM") as ppsB,
                        tc.tile_pool(name="ppsC", bufs=1, space="PSUM") as ppsC,
                    ):
                        na = CH // P

                        def pre_body(iv):
                            hch = pp.tile([P, na, D], f32, tag="hch")
                            nc.sync.dma_start(
                                hch[:], hsh_dram[ds(iv, CH), :].rearrange(
                                    "(a p) d -> p a d", p=P))
                            pT = ppsA.tile([P, CH], f32, tag="pT")
                            for b in range(na):
                                nc.tensor.transpose(
                                    pT[:, b * P:(b + 1) * P], hch[:, b, :], ident[:])
                            hT = pp.tile([P, CH], f32, tag="hT")
                            nc.vector.tensor_copy(hT[:], pT[:])
                            phw = ppsB.tile([P, CH], f32, tag="phw")
                            nc.tensor.matmul(phw[:], Wg_sb[:], hT[:], start=True, stop=True)
                            hwT = pp.tile([P, CH], f32, tag="hwT")
                            nc.vector.tensor_copy(hwT[:], phw[:])
                            pas = ppsC.tile([2, CH], f32, tag="pas")
                            nc.tensor.matmul(pas[:], a2_sb[:], hwT[:], start=True, stop=True)
                            phw2 = ppsA.tile([P, CH], f32, tag="phw2")
                            for b in range(na):
                                nc.tensor.transpose(
                                    phw2[:, b * P:(b + 1) * P],
                                    hwT[:, b * P:(b + 1) * P], ident[:])
                            asv = pp.tile([2, CH], f32, tag="asv")
                            nc.vector.tensor_copy(asv[:], pas[:])
                            pasT = ppsC.tile([P, 2 * na], f32, tag="pasT")
                            for b in range(na):
                                nc.tensor.transpose(
                                    pasT[:, b * 2:(b + 1) * 2],
                                    asv[:, b * P:(b + 1) * P], ident[:2, :2])
                            stg = pp.tile([P, na, DW], bf16, tag="stg")
                            nc.vector.tensor_copy(
                                stg[:, :, 0:D], phw2[:].rearrange("p (a d) -> p a d", a=na))
                            nc.scalar.activation(
                                stg[:, :, D:D + 1],
                                pasT[:].rearrange("p (a t) -> p a t", a=na)[:, :, 0:1],
                                AT.Copy)
                            nc.vector.memset(stg[:, :, D + 1:DW], 0.0)
                            nc.sync.dma_start(
                                cc_bf[ds(iv, CH), :].rearrange("(a p) d -> p a d", p=P),
                                stg[:])
                            nc.sync.dma_start(ad_dram[0:1, ds(iv, CH)], asv[1:2, :])
                            if t_spad:
                                nc.sync.dma_start(
                                    ad2[ds(iv, CH), :].rearrange("n d -> d n"),
                                    asv[1:2, :])
                        tc.For_i_unrolled(0, SH_PAD, CH, pre_body, max_unroll=4)
                        if ad_pad > SH_PAD:
                            zr = pp.tile([1, ad_pad - SH_PAD], f32)
                            nc.vector.memset(zr[:], 0.0)
                            nc.sync.dma_start(ad_dram[0:1, SH_PAD:ad_pad], zr[:])

                    nc.gpsimd.collective_compute(
                        "AllGather", OP.bypass, replica_groups=rg,
                        ins=[cc_bf[:, :]], outs=[hw_aug[:, :]])

                    # ---- edge loop ----
                    with (
                        tc.tile_pool(name="eb", bufs=6) as eb,
                        tc.tile_pool(name="ebR", bufs=2, space="PSUM") as ebR,
                        tc.tile_pool(name="ebo", bufs=6, space="PSUM") as ebo,
                    ):
                        for i0 in range(0, t_pad, G):
                            Rp = ebR.tile([P, gk], f32, tag="Rp")
                            nc.tensor.matmul(Rp[:], one_row[:],
                                             adb_sb[0:1, ds(i0 * k, gk)],
                                             start=True, stop=True)
                            Rb = eb.tile([P, gk], bf16, tag="Rb")
                            nc.vector.tensor_copy(Rb[:], Rp[:])
                            msgb = eb.tile([P, G, DW], bf16, tag="msgb")
                            for j in range(G):
                                nc.gpsimd.indirect_dma_start(
                                    out=msgb[:, j, :], out_offset=None, in_=hw_aug[:],
                                    in_offset=IndirectOffsetOnAxis(
                                        ap=srcA_sb[:, i0 + j:i0 + j + 1], axis=0))
                            Bt = eb.tile([P, G, k], bf16, tag="Bt")
                            nc.vector.tensor_tensor(
                                out=Bt[:],
                                in0=Rb[:].rearrange("p (g s) -> p g s", g=G),
                                in1=msgb[:, :, D:D + 1].to_broadcast([P, G, k]),
                                op=OP.add)
                            lr = eb.tile([P, G, k], bf16, tag="lr")
                            nc.vector.tensor_scalar_mul(lr[:], Bt[:], NEG)
                            nc.vector.tensor_tensor(out=lr[:], in0=lr[:], in1=Bt[:], op=OP.max)
                            M0 = eb.tile([P, G, k], bf16, tag="M0")
                            nc.scalar.activation(M0[:], lr[:], AT.Exp)
                            Sel = eb.tile([P, G, k], bf16, tag="Sel")
                            nc.vector.tensor_tensor(
                                out=Sel[:],
                                in0=slotW_sb[:, i0:i0 + G].rearrange(
                                    "p (g o) -> p g o", o=1).to_broadcast([P, G, k]),
                                in1=iotaB[:].rearrange("p (g s) -> p g s", g=G),
                                op=OP.is_equal)
                            nc.vector.tensor_tensor(out=Sel[:], in0=Sel[:], in1=M0[:], op=OP.mult)
                            for j in range(G):
                                po = ebo.tile([k, D + 1], f32, tag="po")
                                nc.tensor.matmul(po[:, 0:D], Sel[:, j, :],
                                                 msgb[:, j, 0:D], start=True, stop=True)
                                nc.tensor.matmul(po[:, D:D + 1], Sel[:, j, :],
                                                 ones_col[:], start=True, stop=True)
                                st = eb.tile([k, D + 1], f32, tag="st")
                                if j % 2 == 0:
                                    nc.vector.tensor_copy(st[:], po[:])
                                else:
                                    nc.scalar.activation(st[:], po[:], AT.Copy)
                                nc.sync.dma_start(
                                    outbuf[ds(i0 * k + j * k, k), :], st[:])

                    # ---- spill pass: overflow edges, indirect scatter-add ----
                    if t_spad:
                        with (
                            tc.tile_pool(name="sp", bufs=3) as spp,
                            tc.tile_pool(name="spo", bufs=2, space="PSUM") as spo,
                        ):
                            with tc.For_i(0, t_spad, 1) as s0:
                                sxt = spp.tile([P, 1], mybir.dt.int32, tag="sxt")
                                nc.sync.dma_start(sxt[:], srcSp[:, ds(s0, 1)])
                                dxt = spp.tile([P, 1], mybir.dt.int32, tag="dxt")
                                nc.sync.dma_start(dxt[:], dstSp[:, ds(s0, 1)])
                                rxt = spp.tile([P, 1], mybir.dt.int32, tag="rxt")
                                nc.sync.dma_start(rxt[:], rixSp[:, ds(s0, 1)])
                                slb1 = spp.tile([P, 1], bf16, tag="slb1")
                                nc.sync.dma_start(slb1[:], slotSp[:, ds(s0, 1)])
                                msg1 = spp.tile([P, DW], bf16, tag="msg1")
                                nc.gpsimd.indirect_dma_start(
                                    out=msg1[:], out_offset=None, in_=hw_aug[:],
                                    in_offset=IndirectOffsetOnAxis(
                                        ap=sxt[:, 0:1], axis=0))
                                adc = spp.tile([P, 1], f32, tag="adc")
                                nc.gpsimd.indirect_dma_start(
                                    out=adc[:], out_offset=None, in_=ad2[:],
                                    in_offset=IndirectOffsetOnAxis(
                                        ap=dxt[:, 0:1], axis=0))
                                Bt1 = spp.tile([P, 1], bf16, tag="Bt1")
                                nc.vector.tensor_copy(Bt1[:], adc[:])
                                nc.vector.tensor_tensor(
                                    out=Bt1[:], in0=Bt1[:],
                                    in1=msg1[:, D:D + 1], op=OP.add)
                                lr1 = spp.tile([P, 1], bf16, tag="lr1")
                                nc.vector.tensor_scalar_mul(lr1[:], Bt1[:], NEG)
                                nc.vector.tensor_tensor(
                                    out=lr1[:], in0=lr1[:], in1=Bt1[:], op=OP.max)
                                M1 = spp.tile([P, 1], bf16, tag="M1")
                                nc.scalar.activation(M1[:], lr1[:], AT.Exp)
                                Sel1 = spp.tile([P, P], bf16, tag="Sel1")
                                nc.vector.tensor_tensor(
                                    out=Sel1[:],
                                    in0=slb1[:].to_broadcast([P, P]),
                                    in1=iotaS[:], op=OP.is_equal)
                                nc.vector.tensor_tensor(
                                    out=Sel1[:], in0=Sel1[:],
                                    in1=M1[:].to_broadcast([P, P]), op=OP.mult)
                                po1 = spo.tile([P, D + 1], f32, tag="po1")
                                nc.tensor.matmul(po1[:, 0:D], Sel1[:],
                                                 msgb_sp := msg1[:, 0:D],
                                                 start=True, stop=True)
                                nc.tensor.matmul(po1[:, D:D + 1], Sel1[:],
                                                 ones_col[:], start=True, stop=True)
                                st1 = spp.tile([P, D + 1], f32, tag="st1")
                                nc.vector.tensor_copy(st1[:], po1[:])
                                nc.gpsimd.indirect_dma_start(
                                    out=outbuf[:], out_offset=IndirectOffsetOnAxis(
                                        ap=rxt[:, 0:1], axis=0),
                                    in_=st1[:], in_offset=None,
                                    compute_op=OP.add)

                    # ---- finalize: conv = (sum of 4 outbufs)/denom + bg ----
                    with (
                        tc.tile_pool(name="fp", bufs=3) as fp,
                        tc.tile_pool(name="facc", bufs=1) as facc,
                        tc.tile_pool(name="fps", bufs=2, space="PSUM") as fps,
                    ):
                        acc = facc.tile([fr, 2], f32)
                        nc.vector.memset(acc[:], 0.0)

                        def fin_body(iv):
                            ob = fp.tile([fr, D + 1], f32, tag="ob")
                            nc.sync.dma_start(ob[:], outbuf[ds(iv, fr), :])
                            rcp = fp.tile([fr, 1], f32, tag="rcp")
                            nc.vector.reciprocal(rcp[:], ob[:, D:D + 1])
                            cv = fp.tile([fr, D], f32, tag="cv")
                            nc.vector.tensor_tensor(
                                out=cv[:], in0=ob[:, 0:D],
                                in1=rcp[:].to_broadcast([fr, D]), op=OP.mult)
                            nc.vector.tensor_tensor(
                                out=cv[:], in0=cv[:], in1=bg_sb[:fr, :], op=OP.add)
                            s1 = fp.tile([fr, 1], f32, tag="s1")
                            nc.vector.tensor_reduce(
                                out=s1[:], in_=cv[:], axis=mybir.AxisListType.X, op=OP.add)
                            sqv = fp.tile([fr, D], f32, tag="sqv")
                            s2 = fp.tile([fr, 1], f32, tag="s2")
                            nc.scalar.activation(sqv[:], cv[:], AT.Square, accum_out=s2[:])
                            nc.vector.tensor_tensor(
                                out=acc[:, 0:1], in0=acc[:, 0:1], in1=s1[:], op=OP.add)
                            nc.vector.tensor_tensor(
                                out=acc[:, 1:2], in0=acc[:, 1:2], in1=s2[:], op=OP.add)
                            nc.sync.dma_start(conv_dram[ds(iv, fr), :], cv[:])
                        tc.For_i_unrolled(0, nsh, fr, fin_body, max_unroll=4)

                        pst = fps.tile([1, 2], f32)
                        nc.tensor.matmul(pst[:], ones_col_f[:fr, :], acc[:], start=True, stop=True)
                        stt = sml.tile([1, 2], f32, tag="stt")
                        nc.vector.tensor_copy(stt[:], pst[:])
                        nc.sync.dma_start(st_in[:, :], stt[:])

                    nc.gpsimd.collective_compute(
                        "AllReduce", OP.add, replica_groups=rg,
                        ins=[st_in[:, :]], outs=[st_out[:, :]])

                    # ---- stats -> scale/shift, apply LN + relu + residual ----
                    with (
                        tc.tile_pool(name="ap", bufs=3) as apl,
                        tc.tile_pool(name="aps", bufs=2, space="PSUM") as aps,
                    ):
                        sto = sml.tile([1, 2], f32, tag="sto")
                        nc.sync.dma_start(sto[:], st_out[:, :])
                        mn = sml.tile([1, 1], f32, tag="mn")
                        nc.vector.tensor_scalar_mul(mn[:], sto[:, 0:1], nd_inv)
                        ms = sml.tile([1, 1], f32, tag="ms")
                        nc.vector.tensor_scalar_mul(ms[:], sto[:, 1:2], nd_inv)
                        m2 = sml.tile([1, 1], f32, tag="m2")
                        nc.vector.tensor_tensor(out=m2[:], in0=mn[:], in1=mn[:], op=OP.mult)
                        vr = sml.tile([1, 1], f32, tag="vr")
                        nc.vector.tensor_tensor(out=vr[:], in0=ms[:], in1=m2[:], op=OP.subtract)
                        nc.vector.tensor_scalar_add(vr[:], vr[:], EPS)
                        sd = sml.tile([1, 1], f32, tag="sd")
                        nc.scalar.activation(sd[:], vr[:], AT.Sqrt)
                        rs = sml.tile([1, 1], f32, tag="rs")
                        nc.vector.reciprocal(rs[:], sd[:])
                        nmr = sml.tile([1, 1], f32, tag="nmr")
                        nc.vector.tensor_tensor(out=nmr[:], in0=mn[:], in1=rs[:], op=OP.mult)
                        nc.vector.tensor_scalar_mul(nmr[:], nmr[:], -1.0)
                        pk = sml.tile([1, 2], f32, tag="pk")
                        nc.vector.tensor_copy(pk[:, 0:1], rs[:])
                        nc.vector.tensor_copy(pk[:, 1:2], nmr[:])
                        pbc = aps.tile([P, 2], f32)
                        nc.tensor.matmul(pbc[:], one_row_f[:], pk[:], start=True, stop=True)
                        bc = sml.tile([P, 2], f32, tag="bc")
                        nc.vector.tensor_copy(bc[:], pbc[:])

                        def app_body(iv):
                            cv = apl.tile([fr, D], f32, tag="acv")
                            nc.sync.dma_start(cv[:], conv_dram[ds(iv, fr), :])
                            tt = apl.tile([fr, D], f32, tag="att")
                            nc.vector.tensor_scalar(
                                out=tt[:], in0=cv[:], scalar1=bc[:fr, 0:1],
                                scalar2=bc[:fr, 1:2], op0=OP.mult, op1=OP.add)
                            nc.vector.tensor_tensor(
                                out=tt[:], in0=tt[:], in1=lnw_sb[:fr, :], op=OP.mult)
                            nc.vector.tensor_tensor(
                                out=tt[:], in0=tt[:], in1=lnb_sb[:fr, :], op=OP.add)
                            nc.vector.tensor_scalar_max(tt[:], tt[:], 0.0)
                            hin = apl.tile([fr, D], f32, tag="hin")
                            nc.sync.dma_start(hin[:], hsh_dram[ds(iv, fr), :])
                            nc.vector.tensor_tensor(
                                out=tt[:], in0=tt[:], in1=hin[:], op=OP.add)
                            nc.sync.dma_start(hsh_dram[ds(iv, fr), :], tt[:])
                        tc.For_i_unrolled(0, nsh, fr, app_body, max_unroll=4)

            # ---------------- decoder ----------------
            with (
                tc.tile_pool(name="dp", bufs=3) as dp,
                tc.tile_pool(name="dacc", bufs=1) as dac,
                tc.tile_pool(name="dps", bufs=2, space="PSUM") as dps,
            ):
                dacc = dac.tile([1, 1], f32)
                nc.vector.memset(dacc[:], 0.0)

                def dec_body(iv):
                    ch = dp.tile([fr, D], f32, tag="ch")
                    nc.sync.dma_start(ch[:], hsh_dram[ds(iv, fr), :])
                    pT = dps.tile([D, fr], f32, tag="dpT")
                    nc.tensor.transpose(pT[:], ch[:], ident[:fr, :fr])
                    hT = dp.tile([D, fr], f32, tag="hT")
                    nc.vector.tensor_copy(hT[:], pT[:])
                    pz = dps.tile([1, fr], f32, tag="pz")
                    nc.tensor.matmul(pz[:], decW_sb[:], hT[:], start=True, stop=True)
                    zz = dp.tile([1, fr], f32, tag="zz")
                    zs = dp.tile([1, 1], f32, tag="zs")
                    nc.scalar.activation(zz[:], pz[:], AT.Sigmoid,
                                         bias=decb_sb[:], accum_out=zs[:])
                    nc.vector.tensor_tensor(out=dacc[:], in0=dacc[:], in1=zs[:], op=OP.add)
                tc.For_i_unrolled(0, nsh, fr, dec_body, max_unroll=4)
                nc.sync.dma_start(outp[:, :], dacc[:])

    nc.finalize()
    return nc



def _get_nc(nsh, fr, k, t_pad, t_spad, ncores):
    key = (nsh, fr, k, t_pad, t_spad, ncores)
    if key not in _CACHE:
        _CACHE[key] = _build(nsh, fr, k, t_pad, t_spad, ncores)
    return _CACHE[key]


def _prepare(x, edge_index, enc_W, enc_b, Wg, a_src, a_dst, bg, ln_w, ln_b,
             dec_W, dec_b):
    x = np.asarray(x, np.float32)
    n_full = x.shape[0]
    nsh = n_full // NC
    fr = next(f for f in range(min(P, nsh), 0, -1) if nsh % f == 0)
    ei = np.asarray(edge_index)
    loop = np.arange(n_full, dtype=ei.dtype)
    src = np.concatenate([ei[0], loop])
    dst = np.concatenate([ei[1], loop])
    cores, (k, t_pad, t_spad) = _prep(src, dst, n_full, nsh)

    enc_b = np.asarray(enc_b, np.float32)
    Wg = np.asarray(Wg, np.float32)
    a2 = np.stack([np.asarray(a_src, np.float32),
                   np.asarray(a_dst, np.float32)], axis=2)
    bg_r = np.broadcast_to(np.asarray(bg, np.float32)[:, None, :], (L, P, D)).copy()
    lnw_r = np.broadcast_to(np.asarray(ln_w, np.float32)[:, None, :], (L, P, D)).copy()
    lnb_r = np.broadcast_to(np.asarray(ln_b, np.float32)[:, None, :], (L, P, D)).copy()
    encb_r = np.broadcast_to(enc_b[None, :], (P, D)).copy()
    decW_h = np.asarray(dec_W, np.float32).reshape(D, 1)
    decb_h = np.asarray(dec_b, np.float32).reshape(1, 1)

    nc = _get_nc(nsh, fr, k, t_pad, t_spad, NC)
    in_maps = []
    for c in range(NC):
        m = {
            "xs": np.ascontiguousarray(x[c * nsh:(c + 1) * nsh]),
            "encW": np.ascontiguousarray(np.asarray(enc_W, np.float32)),
            "encb": encb_r, "WgP": Wg, "a2P": a2, "bgP": bg_r,
            "lnwP": lnw_r, "lnbP": lnb_r, "decW": decW_h, "decb": decb_h,
        }
        m["srcA"] = cores[c]["srcA"]
        m["slotW"] = cores[c]["slotW"]
        m["iotaP"] = np.broadcast_to(
            np.tile(np.arange(k, dtype=np.float32), G)[None, :],
            (P, G * k)).astype(ml_dtypes.bfloat16).copy()
        if t_spad:
            m["srcS"] = cores[c]["srcS"]
            m["slotS"] = cores[c]["slotS"]
            m["dstS"] = cores[c]["dstS"]
            m["rixS"] = cores[c]["rixS"]
            m["iotaS"] = np.broadcast_to(
                np.arange(P, dtype=np.float32)[None, :],
                (P, P)).astype(ml_dtypes.bfloat16).copy()
        in_maps.append(m)
    return nc, in_maps


def kernel(x, edge_index, enc_W, enc_b, Wg, a_src, a_dst, bg, ln_w, ln_b,
           dec_W, dec_b):
    from concourse.bass_utils import run_bass_kernel_spmd

    nc, in_maps = _prepare(x, edge_index, enc_W, enc_b, Wg, a_src, a_dst,
                           bg, ln_w, ln_b, dec_W, dec_b)
    res = run_bass_kernel_spmd(nc, in_maps, list(range(NC))).results
    total = np.float32(sum(float(res[c]["outp"][0, 0]) for c in range(NC)))
    return np.array([total], np.float32)
